# revision 43
# baseline (speedup 1.0000x reference)
"""v4: fp16 conv1 + bf16-single conv2, both column-tiled pairs (15 MM slots
per 2 spatial tiles).

conv1 (fp16, rel err ~2^-11 on inputs):
  - kd in {0,1} packed into K=128 via xq tiles (lo=plane s, hi=s+1): 9 MMs
  - kd=2: kh in {0,1} packed via xc tiles (lo=plane s+2, hi=s+2 shifted one
    row): 3 MMs K=128; kh=2 leftover: 3 MMs K=64.
  - two spatial row-tiles run concurrently via tile_position=(0,64).
conv2 (bf16 single weights, scaled w2*s2[out]/s1[in]; activations are exact
  integers in bf16): identical 15-slot structure over qp/qc tiles.

Epilogues: conv1 ACT(relu+bias) + 2 DVE magic-round ops -> bf16 frame stage;
frame-batched DMAs scatter the stage into qp/qc pair tiles (4 dests x 2).
conv2: DVE add(bias+MAGIC)/clamp + add precomputed quantized identity (yx,
host-packed as qx-MAGIC) + ACT relu*inv_s2 -> ostage -> 2 DMAs per frame.
"""

import numpy as np
import ml_dtypes
import concourse.mybir as mybir
from concourse import bacc
from concourse.tile import TileContext
from concourse.bass_utils import run_bass_kernel_spmd

BF16 = mybir.dt.bfloat16
FP16 = mybir.dt.float16
F32 = mybir.dt.float32

MANTISA_BIT = 8.0
MAGIC = 12582912.0

N, C, T, H, W = 8, 64, 16, 56, 56
TP, HP, WP = T + 2, H + 2, W + 2
PLANE = HP * WP
SLICE = H * W
ROWS = 7
NT = ROWS * W  # 392
NTILES = H // ROWS  # 8
NPAIR = NTILES // 2  # 4
FNT = NPAIR * NT  # 1568

_COMPILED = None


def _build():
    nc = bacc.Bacc()
    xpad_d = nc.declare_dram_parameter("xpad", [C, TP, PLANE], FP16, isOutput=False)
    yx_d = nc.declare_dram_parameter("yx", [128, T, FNT], F32, isOutput=False)
    w1p9_d = nc.declare_dram_parameter("w1p9", [128, 9 * 64], FP16, isOutput=False)
    w1c3_d = nc.declare_dram_parameter("w1c3", [128, 3 * 64], FP16, isOutput=False)
    w1kp_d = nc.declare_dram_parameter("w1kp", [128, 64], FP16, isOutput=False)
    w1k1_d = nc.declare_dram_parameter("w1k1", [64, 64], FP16, isOutput=False)
    w2p9_d = nc.declare_dram_parameter("w2p9", [128, 9 * 64], BF16, isOutput=False)
    w2c3_d = nc.declare_dram_parameter("w2c3", [128, 3 * 64], BF16, isOutput=False)
    w2kp_d = nc.declare_dram_parameter("w2kp", [128, 64], BF16, isOutput=False)
    w2k1_d = nc.declare_dram_parameter("w2k1", [64, 64], BF16, isOutput=False)
    coeff_d = nc.declare_dram_parameter("coeff", [128, 8], F32, isOutput=False)
    out_d = nc.declare_dram_parameter("out", [C, T * SLICE], F32, isOutput=True)

    with TileContext(nc) as tc:
        with (
            tc.tile_pool(name="big", bufs=1) as bigpool,
            tc.tile_pool(name="ost", bufs=2) as ostpool,
            tc.tile_pool(name="yx", bufs=2) as yxpool,
            tc.tile_pool(name="small", bufs=3) as spool,
            tc.tile_pool(name="ps1", bufs=4, space="PSUM") as ps1pool,
            tc.tile_pool(name="ps2", bufs=4, space="PSUM") as ps2pool,
        ):
            w1p9 = bigpool.tile([128, 9 * 64], FP16, tag="w1p9")
            nc.sync.dma_start(out=w1p9[:], in_=w1p9_d[:])
            w1c3 = bigpool.tile([128, 3 * 64], FP16, tag="w1c3")
            nc.sync.dma_start(out=w1c3[:], in_=w1c3_d[:])
            w1kp = bigpool.tile([128, 64], FP16, tag="w1kp")
            nc.sync.dma_start(out=w1kp[:], in_=w1kp_d[:])
            w1k1 = bigpool.tile([64, 64], FP16, tag="w1k1")
            nc.sync.dma_start(out=w1k1[:], in_=w1k1_d[:])
            w2p9 = bigpool.tile([128, 9 * 64], BF16, tag="w2p9")
            nc.sync.dma_start(out=w2p9[:], in_=w2p9_d[:])
            w2c3 = bigpool.tile([128, 3 * 64], BF16, tag="w2c3")
            nc.sync.dma_start(out=w2c3[:], in_=w2c3_d[:])
            w2kp = bigpool.tile([128, 64], BF16, tag="w2kp")
            nc.sync.dma_start(out=w2kp[:], in_=w2kp_d[:])
            w2k1 = bigpool.tile([64, 64], BF16, tag="w2k1")
            nc.sync.dma_start(out=w2k1[:], in_=w2k1_d[:])
            coeff = bigpool.tile([128, 8], F32, tag="coeff")
            nc.sync.dma_start(out=coeff[:], in_=coeff_d[:])

            b1s1 = coeff[:, 0:1]
            bM2 = coeff[:, 1:2]
            inv_s2 = coeff[:, 2:3]

            # persistent rings
            xq = [bigpool.tile([128, PLANE], FP16, tag=f"xq{i}", name=f"xq{i}")
                  for i in range(3)]
            xc = [bigpool.tile([128, PLANE], FP16, tag=f"xc{i}", name=f"xc{i}")
                  for i in range(3)]
            xs = [bigpool.tile([128, PLANE], FP16, tag=f"xs{i}", name=f"xs{i}")
                  for i in range(3)]
            qp = [bigpool.tile([128, PLANE], BF16, tag=f"qp{i}", name=f"qp{i}")
                  for i in range(5)]
            qc = [bigpool.tile([128, PLANE], BF16, tag=f"qc{i}", name=f"qc{i}")
                  for i in range(3)]
            qs = [bigpool.tile([128, PLANE], BF16, tag=f"qs{i}", name=f"qs{i}")
                  for i in range(3)]
            # stage: half-frame band in padded-row layout (28 rows x 58),
            # border cols pre-zeroed so flush DMAs are fully contiguous
            BAND = (H // 2) * WP  # 1624
            stg = [bigpool.tile([128, BAND], BF16, tag=f"stg{i}", name=f"stg{i}")
                   for i in range(2)]
            for s_ in stg:
                sv_ = s_[:].rearrange("p (r w) -> p r w", w=WP)
                nc.gpsimd.memset(sv_[:, :, 0], 0.0)
                nc.gpsimd.memset(sv_[:, :, WP - 1], 0.0)

            def border_zero(v):
                vv = v.rearrange("p (h w) -> p h w", w=WP)
                nc.gpsimd.memset(vv[:, 0, :], 0.0)
                nc.gpsimd.memset(vv[:, HP - 1, :], 0.0)
                nc.gpsimd.memset(vv[:, 1 : HP - 1, 0], 0.0)
                nc.gpsimd.memset(vv[:, 1 : HP - 1, WP - 1], 0.0)

            for tl in qp + qc + qs:
                border_zero(tl[:])
            nc.gpsimd.memset(qp[0][0:64, :], 0.0)  # plane 0 is all-pad

            def load_x(s):
                a = xq[s % 3]
                nc.sync.dma_start(out=a[0:64, :], in_=xpad_d[:, s, :])
                nc.sync.dma_start(out=a[64:128, :], in_=xpad_d[:, s + 1, :])
                b = xc[s % 3]
                nc.sync.dma_start(out=b[0:64, :], in_=xpad_d[:, s + 2, :])
                nc.sync.dma_start(
                    out=b[64:128, 0 : PLANE - WP], in_=xpad_d[:, s + 2, WP:PLANE]
                )
                c_ = xs[s % 3]
                nc.sync.dma_start(out=c_[0:64, :], in_=xpad_d[:, s + 2, :])
                nc.sync.dma_start(
                    out=c_[64:128, 0 : PLANE - 1], in_=xpad_d[:, s + 2, 1:PLANE]
                )
                c_ = xs[s % 3]
                nc.sync.dma_start(out=c_[0:64, :], in_=xpad_d[:, s + 2, :])
                nc.sync.dma_start(
                    out=c_[64:128, 0 : PLANE - 1], in_=xpad_d[:, s + 2, 1:PLANE]
                )

            def pview(ap):
                return ap.rearrange("p (h w) -> p h w", w=WP)

            def conv_block(ps, wp9, wc3, wkp, wk1, xqv, xcv, xsv, xc64v,
                           r0A, r0B, skip_kd2):
                nslots = 9 if skip_kd2 else 14
                s = 0
                for kh in range(3):
                    for kw in range(3):
                        w_ = wp9[:, 64 * (3 * kh + kw) : 64 * (3 * kh + kw) + 64]
                        first, last = s == 0, s == nslots - 1
                        nc.tensor.matmul(
                            ps[0:64, :], w_,
                            xqv[:, r0A + kh : r0A + kh + ROWS, kw : kw + W],
                            start=first, stop=last, skip_group_check=True,
                        )
                        nc.tensor.matmul(
                            ps[64:128, :], w_,
                            xqv[:, r0B + kh : r0B + kh + ROWS, kw : kw + W],
                            start=first, stop=last, tile_position=(0, 64),
                            skip_group_check=True,
                        )
                        s += 1
                if skip_kd2:
                    return
                if kp is not None:
                    wkp_, wk1_ = kp
                    for kw in range(3):
                        w_ = wc3[:, 64 * kw : 64 * kw + 64]
                        nc.tensor.matmul(
                            ps[0:64, :], w_, xcv[:, r0A : r0A + ROWS, kw : kw + W],
                            start=False, stop=False, skip_group_check=True,
                        )
                        nc.tensor.matmul(
                            ps[64:128, :], w_, xcv[:, r0B : r0B + ROWS, kw : kw + W],
                            start=False, stop=False, tile_position=(0, 64),
                            skip_group_check=True,
                        )
                    nc.tensor.matmul(
                        ps[0:64, :], wkp_[:], xsv[:, r0A + 2 : r0A + 2 + ROWS, 0:W],
                        start=False, stop=False, skip_group_check=True,
                    )
                    nc.tensor.matmul(
                        ps[64:128, :], wkp_[:], xsv[:, r0B + 2 : r0B + 2 + ROWS, 0:W],
                        start=False, stop=False, tile_position=(0, 64),
                        skip_group_check=True,
                    )
                    nc.tensor.matmul(
                        ps[0:64, :], wk1_[:],
                        xc64v[:, r0A + 2 : r0A + 2 + ROWS, 2 : 2 + W],
                        start=False, stop=True, skip_group_check=True,
                    )
                    nc.tensor.matmul(
                        ps[64:128, :], wk1_[:],
                        xc64v[:, r0B + 2 : r0B + 2 + ROWS, 2 : 2 + W],
                        start=False, stop=True, tile_position=(0, 64),
                        skip_group_check=True,
                    )
                    return
                for kw in range(3):
                    w_ = wc3[:, 64 * kw : 64 * kw + 64]
                    nc.tensor.matmul(
                        ps[0:64, :], w_, xcv[:, r0A : r0A + ROWS, kw : kw + W],
                        start=False, stop=False, skip_group_check=True,
                    )
                    nc.tensor.matmul(
                        ps[64:128, :], w_, xcv[:, r0B : r0B + ROWS, kw : kw + W],
                        start=False, stop=False, tile_position=(0, 64),
                        skip_group_check=True,
                    )
                # (kh=2, kw=0) + (kh=2, kw=1) packed via col-shifted pair tile
                nc.tensor.matmul(
                    ps[0:64, :], wkp[:], xsv[:, r0A + 2 : r0A + 2 + ROWS, 0:W],
                    start=False, stop=False, skip_group_check=True,
                )
                nc.tensor.matmul(
                    ps[64:128, :], wkp[:], xsv[:, r0B + 2 : r0B + 2 + ROWS, 0:W],
                    start=False, stop=False, tile_position=(0, 64),
                    skip_group_check=True,
                )
                # (kh=2, kw=2) leftover, K=64
                nc.tensor.matmul(
                    ps[0:64, :], wk1[:], xc64v[:, r0A + 2 : r0A + 2 + ROWS, 2 : 2 + W],
                    start=False, stop=True, skip_group_check=True,
                )
                nc.tensor.matmul(
                    ps[64:128, :], wk1[:],
                    xc64v[:, r0B + 2 : r0B + 2 + ROWS, 2 : 2 + W],
                    start=False, stop=True, tile_position=(0, 64),
                    skip_group_check=True,
                )

            HH = H // 2  # 28: each PSUM col-tile half covers a contiguous band

            def bandoff(hi_band, shift_up=False):
                # element offset of a 28-row band in a padded plane
                return ((0 if shift_up else 1) + (HH if hi_band else 0)) * WP

            load_x(0)
            load_x(1)
            yxtiles = {}

            for t in range(T + 2):
                if 1 <= t <= T:
                    yxt = yxpool.tile([128, FNT], F32, tag="yx")
                    nc.sync.dma_start(out=yxt[:], in_=yx_d[:, t - 1, :])
                    yxtiles[t - 1] = yxt
                if t < T:
                    if t + 2 < T:
                        load_x(t + 2)
                    xqv = pview(xq[t % 3][:])
                    xcv = pview(xc[t % 3][:])
                    xsv = pview(xs[t % 3][:])
                    xc64v = pview(xc[t % 3][0:64, :])
                    stage = stg[t % 2]
                    sv = stage[:].rearrange("p (r w) -> p r w", w=WP)
                    for p in range(NPAIR):
                        r0A, r0B = p * ROWS, p * ROWS + HH
                        ps = ps1pool.tile([128, NT], F32, tag="ps1")
                        conv_block(ps, w1p9, w1c3, w1kp, w1k1, xqv, xcv, xsv,
                                   xc64v, r0A, r0B, skip_kd2=(t == T - 1))
                        r_sb = spool.tile([128, NT], F32, tag="r1")
                        nc.scalar.activation(
                            r_sb[:], ps[:], mybir.ActivationFunctionType.Relu,
                            bias=b1s1, scale=1.0,
                        )
                        m_sb = spool.tile([128, NT], F32, tag="m1")
                        nc.vector.tensor_scalar(
                            out=m_sb[:], in0=r_sb[:],
                            scalar1=MAGIC, scalar2=MAGIC + 127.0,
                            op0=mybir.AluOpType.add, op1=mybir.AluOpType.min,
                        )
                        nc.vector.tensor_scalar(
                            out=sv[:, p * ROWS : (p + 1) * ROWS, 1 : 1 + W],
                            in0=m_sb[:].rearrange("p (r w) -> p r w", w=W),
                            scalar1=MAGIC, scalar2=None,
                            op0=mybir.AluOpType.subtract,
                        )
                    # frame flush: plane t+1 -> qp[t+1].lo, qp[t].hi, qc[t-1]
                    sA = stage[0:64, :]
                    sB = stage[64:128, :]
                    if t + 1 < T:
                        h = qp[(t + 1) % 5]
                        oo = bandoff(False)
                        nc.sync.dma_start(out=h[0:64, oo : oo + BAND], in_=sA)
                        oo = bandoff(True)
                        nc.sync.dma_start(out=h[0:64, oo : oo + BAND], in_=sB)
                    h = qp[t % 5]
                    oo = bandoff(False)
                    nc.sync.dma_start(out=h[64:128, oo : oo + BAND], in_=sA)
                    oo = bandoff(True)
                    nc.sync.dma_start(out=h[64:128, oo : oo + BAND], in_=sB)
                    if t >= 1 and t - 1 < T - 1:
                        h = qc[(t - 1) % 3]
                        oo = bandoff(False)
                        nc.sync.dma_start(out=h[0:64, oo : oo + BAND], in_=sA)
                        oo = bandoff(True)
                        nc.sync.dma_start(out=h[0:64, oo : oo + BAND], in_=sB)
                        oo = bandoff(False, shift_up=True)
                        nc.sync.dma_start(out=h[64:128, oo : oo + BAND], in_=sA)
                        oo = bandoff(True, shift_up=True)
                        nc.sync.dma_start(out=h[64:128, oo : oo + BAND], in_=sB)
                        h = qs[(t - 1) % 3]
                        oo = bandoff(False)
                        nc.gpsimd.dma_start(out=h[0:64, oo : oo + BAND], in_=sA)
                        oo = bandoff(True)
                        nc.gpsimd.dma_start(out=h[0:64, oo : oo + BAND], in_=sB)
                        oo = bandoff(False) - 1
                        nc.gpsimd.dma_start(out=h[64:128, oo : oo + BAND], in_=sA)
                        oo = bandoff(True) - 1
                        nc.gpsimd.dma_start(out=h[64:128, oo : oo + BAND], in_=sB)

                if t >= 2:
                    u = t - 2
                    skip2 = u == T - 1
                    qpv = pview(qp[u % 5][:])
                    qcv = pview(qc[u % 3][:]) if not skip2 else qpv
                    qsv = pview(qs[u % 3][:]) if not skip2 else qpv
                    qc64v = pview(qc[u % 3][0:64, :]) if not skip2 else qpv
                    yxu = yxtiles.pop(u)
                    ost = ostpool.tile([128, FNT], F32, tag="ost")
                    for p in range(NPAIR):
                        r0A, r0B = p * ROWS, p * ROWS + HH
                        ps = ps2pool.tile([128, NT], F32, tag="ps2")
                        conv_block(ps, w2p9, w2c3, w2kp, w2k1, qpv, qcv, qsv,
                                   qc64v, r0A, r0B, skip_kd2=skip2)
                        a2 = spool.tile([128, NT], F32, tag="a2")
                        nc.scalar.activation(
                            a2[:], ps[:], mybir.ActivationFunctionType.Identity,
                            bias=bM2, scale=1.0,
                        )
                        u2 = spool.tile([128, NT], F32, tag="u2")
                        nc.vector.tensor_scalar(
                            out=u2[:], in0=a2[:],
                            scalar1=MAGIC, scalar2=MAGIC - 127.0,
                            op0=mybir.AluOpType.add, op1=mybir.AluOpType.max,
                        )
                        y2 = spool.tile([128, NT], F32, tag="y2")
                        nc.vector.tensor_scalar(
                            out=y2[:], in0=u2[:],
                            scalar1=MAGIC + 127.0, scalar2=None,
                            op0=mybir.AluOpType.min,
                        )
                        z = spool.tile([128, NT], F32, tag="z")
                        nc.vector.tensor_tensor(
                            out=z[:], in0=y2[:], in1=yxu[:, p * NT : (p + 1) * NT],
                            op=mybir.AluOpType.add,
                        )
                        nc.scalar.activation(
                            ost[:, p * NT : (p + 1) * NT], z[:],
                            mybir.ActivationFunctionType.Relu,
                            bias=0.0, scale=inv_s2,
                        )
                    nc.sync.dma_start(
                        out=out_d[:, u * SLICE : u * SLICE + SLICE // 2],
                        in_=ost[0:64, :],
                    )
                    nc.sync.dma_start(
                        out=out_d[:, u * SLICE + SLICE // 2 : (u + 1) * SLICE],
                        in_=ost[64:128, :],
                    )
    nc.compile()
    return nc


def _host_pack(x, w1, b1, w2, b2, exp1, exp2):
    s1 = np.exp2(MANTISA_BIT - 1.0 - exp1).astype(np.float32)
    s2 = np.exp2(MANTISA_BIT - 1.0 - exp2).astype(np.float32)

    w1f = (w1 * s1[:, None, None, None, None]).astype(np.float32)
    w1t = np.transpose(w1f, (2, 3, 4, 1, 0))  # [kd,kh,kw,i,o]
    w1p9 = np.stack(
        [np.concatenate([w1t[0, kh, kw], w1t[1, kh, kw]], axis=0)
         for kh in range(3) for kw in range(3)]
    )  # [9,128,64]
    w1p9 = np.ascontiguousarray(np.transpose(w1p9, (1, 0, 2))).reshape(128, 9 * 64)
    w1c3 = np.stack(
        [np.concatenate([w1t[2, 0, kw], w1t[2, 1, kw]], axis=0) for kw in range(3)]
    )
    w1c3 = np.ascontiguousarray(np.transpose(w1c3, (1, 0, 2))).reshape(128, 3 * 64)
    w1kp = np.concatenate([w1t[2, 2, 0], w1t[2, 2, 1]], axis=0)
    w1k1 = w1t[2, 2, 2]

    w2f = (w2 * s2[:, None, None, None, None]
           / s1[None, :, None, None, None]).astype(np.float32)
    w2t = np.transpose(w2f, (2, 3, 4, 1, 0))
    w2p9 = np.stack(
        [np.concatenate([w2t[0, kh, kw], w2t[1, kh, kw]], axis=0)
         for kh in range(3) for kw in range(3)]
    )
    w2p9 = np.ascontiguousarray(np.transpose(w2p9, (1, 0, 2))).reshape(128, 9 * 64)
    w2c3 = np.stack(
        [np.concatenate([w2t[2, 0, kw], w2t[2, 1, kw]], axis=0) for kw in range(3)]
    )
    w2c3 = np.ascontiguousarray(np.transpose(w2c3, (1, 0, 2))).reshape(128, 3 * 64)
    w2kp = np.concatenate([w2t[2, 2, 0], w2t[2, 2, 1]], axis=0)
    w2k1 = w2t[2, 2, 2]

    c64 = np.zeros((64, 8), dtype=np.float32)
    c64[:, 0] = b1 * s1
    c64[:, 1] = b2 * s2
    c64[:, 2] = 1.0 / s2
    coeff = np.concatenate([c64, c64], axis=0)

    shared = {
        "w1p9": w1p9.astype(np.float16), "w1c3": w1c3.astype(np.float16),
        "w1kp": np.ascontiguousarray(w1kp).astype(np.float16),
        "w1k1": np.ascontiguousarray(w1k1).astype(np.float16),
        "w2p9": w2p9.astype(ml_dtypes.bfloat16),
        "w2c3": w2c3.astype(ml_dtypes.bfloat16),
        "w2kp": np.ascontiguousarray(w2kp).astype(ml_dtypes.bfloat16),
        "w2k1": np.ascontiguousarray(w2k1).astype(ml_dtypes.bfloat16),
        "coeff": coeff,
    }
    in_maps = []
    for n in range(N):
        xp = np.pad(x[n], ((0, 0), (1, 1), (1, 1), (1, 1))).astype(np.float16)
        m = dict(shared)
        m["xpad"] = np.ascontiguousarray(xp.reshape(C, TP, PLANE))
        qx = np.clip(np.round(x[n] * s2[:, None, None, None]), -127.0, 127.0)
        yxm = (qx - MAGIC).astype(np.float32)  # [C, T, H, W]
        yx = np.concatenate(
            [yxm[:, :, 0 : H // 2, :].reshape(C, T, FNT),
             yxm[:, :, H // 2 :, :].reshape(C, T, FNT)],
            axis=0,
        )
        m["yx"] = np.ascontiguousarray(yx)
        in_maps.append(m)
    return in_maps


def kernel(x, w1, b1, w2, b2, exp1, exp2):
    global _COMPILED
    x = np.asarray(x, dtype=np.float32)
    w1 = np.asarray(w1, dtype=np.float32)
    b1 = np.asarray(b1, dtype=np.float32)
    w2 = np.asarray(w2, dtype=np.float32)
    b2 = np.asarray(b2, dtype=np.float32)
    exp1 = np.asarray(exp1, dtype=np.float32)
    exp2 = np.asarray(exp2, dtype=np.float32)
    if _COMPILED is None:
        _COMPILED = _build()
    in_maps = _host_pack(x, w1, b1, w2, b2, exp1, exp2)
    res = run_bass_kernel_spmd(_COMPILED, in_maps, core_ids=list(range(N)))
    out = np.stack([res.results[i]["out"].reshape(C, T, H, W) for i in range(N)])
    return out.astype(np.float32)
